# revision 10
# baseline (speedup 1.0000x reference)
"""CrossAttention kernel for 8 TRN2 NeuronCores.

Sharding: 8 cores = 4 batches x 2 query-halves (zero communication).
Each core computes all 16 heads for its 1024 queries:
  q^T = Wq^T x^T, k^T = Wk^T ctx^T, v = ctx Wv          (bf16 matmuls)
  scores^T[kpos, q] = k^T.T q^T / 8                      (K=64 per head)
  exp on ScalarE straight from PSUM (no max subtraction; |scores| ~ N(0,1))
  attn_out^T[d, q] + denominators via ones-augmented V (M=65 matmuls)
  normalize by 1/denominator (DMA partition-broadcast), out-proj with
  bias folded in as a K=1 accumulating matmul.
"""

import sys

for _p in ("/opt/trn_rl_repo", "/root/.axon_site/_ro/trn_rl_repo"):
    if _p not in sys.path:
        sys.path.append(_p)

import numpy as np

import concourse.bass as bass
import concourse.tile as tile
from concourse import bacc, mybir
from concourse.bass_utils import run_bass_kernel_spmd

F32 = mybir.dt.float32
BF16 = mybir.dt.bfloat16
EXP = mybir.ActivationFunctionType.Exp
MULT = mybir.AluOpType.mult

P = 128
B, NQ_FULL, DQ = 4, 2048, 1024
NK, DC = 1024, 768
H, DH = 16, 64
INNER = H * DH  # 1024
NT = 1024  # local queries per core
N_CORES = 8

KQ = DQ // P      # 8  k-subtiles for q-proj
KC = DC // P      # 6  k-subtiles for k/v-proj
KI = INNER // P   # 8  inner subtiles
TB = NT // P      # 8  token blocks
KB = NK // P      # 8  kpos blocks
SCALE = 1.0 / np.sqrt(DH)


def _bcast_part(ap, nparts):
    """AP view broadcasting a 1-partition AP across `nparts` partitions (DMA only)."""
    return bass.AP(tensor=ap.tensor, offset=ap.offset,
                   ap=[[0, nparts]] + list(ap.ap[1:]))


def build():
    nc = bacc.Bacc("TRN2", target_bir_lowering=False, debug=False,
                   enable_asserts=False, num_devices=N_CORES)

    x_d = nc.dram_tensor("x", [NT, DQ], F32, kind="ExternalInput")
    ctx_d = nc.dram_tensor("ctx", [NK, DC], F32, kind="ExternalInput")
    wq_d = nc.dram_tensor("wq", [DQ, INNER], F32, kind="ExternalInput")
    wk_d = nc.dram_tensor("wk", [DC, INNER], F32, kind="ExternalInput")
    wv_d = nc.dram_tensor("wv", [DC, INNER], F32, kind="ExternalInput")
    wo_d = nc.dram_tensor("wo", [INNER, DQ], F32, kind="ExternalInput")
    bo_d = nc.dram_tensor("bo", [DQ], F32, kind="ExternalInput")
    out_d = nc.dram_tensor("out", [NT, DQ], F32, kind="ExternalOutput")

    with tile.TileContext(nc) as tc:
        with (
            tc.tile_pool(name="persist", bufs=1) as persist,
            tc.tile_pool(name="psA", bufs=2, space="PSUM") as psA,
            tc.tile_pool(name="psV", bufs=2, space="PSUM") as psV,
        ):
            # ---------------- phase 1: load/cast/transpose inputs ----------------
            qT = persist.tile([P, KI, NT], BF16)     # [inner, q]
            kT = persist.tile([P, KI, NK], BF16)     # [inner, kpos]
            vA = persist.tile([P, KB, H, DH + 1], BF16)  # [kpos, (head, d|1)]
            attnT = persist.tile([P, KI, NT], BF16)  # [inner, q]
            wo_b = persist.tile([P, KI, DQ], BF16)
            bo_b = persist.tile([1, DQ], BF16)
            ones_b = persist.tile([1, P], BF16)
            nc.vector.memset(ones_b[:], 1.0)
            nc.vector.memset(vA[:, :, :, DH:DH + 1], 1.0)

            with tc.tile_pool(name="stage", bufs=1) as stage, \
                 tc.tile_pool(name="stage2", bufs=3) as stage2:
                xT = stage.tile([P, KQ, NT], BF16, tag="xT")
                cT = stage.tile([P, KC, NK], BF16, tag="cT")
                wq_b = stage.tile([P, KQ, INNER], BF16, tag="wq")
                wk_b = stage.tile([P, KC, INNER], BF16, tag="wk")
                wv_b = stage.tile([P, KC, INNER], BF16, tag="wv")

                # weights: load f32, cast to bf16 (alternate engines)
                for name, dst, src, nsub in (
                    ("wq", wq_b, wq_d, KQ), ("wk", wk_b, wk_d, KC),
                    ("wv", wv_b, wv_d, KC), ("wo", wo_b, wo_d, KI),
                ):
                    src3 = src.ap().rearrange("(o p) m -> p o m", p=P)
                    for ko in range(nsub):
                        wf = stage2.tile([P, INNER], F32, tag="ldf")
                        nc.sync.dma_start(wf[:], src3[:, ko])
                        eng = nc.vector if ko % 2 == 0 else nc.gpsimd
                        eng.tensor_copy(dst[:, ko], wf[:])

                bo_f = stage.tile([1, DQ], F32, tag="bo")
                nc.sync.dma_start(bo_f[:], bo_d.ap()[None, :])
                nc.vector.tensor_copy(bo_b[:], bo_f[:])

                # x / ctx: load natural, cast, DMA-transpose into xT / cT
                x3 = x_d.ap().rearrange("(t p) d -> p t d", p=P)
                c3 = ctx_d.ap().rearrange("(t p) d -> p t d", p=P)
                for t in range(TB):
                    xf = stage2.tile([P, DQ], F32, tag="ldf")
                    nc.sync.dma_start(xf[:], x3[:, t])
                    xb = stage2.tile([P, DQ], BF16, tag="castb")
                    eng = nc.vector if t % 2 == 0 else nc.gpsimd
                    eng.tensor_copy(xb[:], xf[:])
                    for ko in range(KQ):
                        nc.sync.dma_start_transpose(
                            xT[:, ko, t * P:(t + 1) * P],
                            xb[:, ko * P:(ko + 1) * P])
                for t in range(KB):
                    cf = stage2.tile([P, DQ], F32, tag="ldf")
                    nc.sync.dma_start(cf[:, :DC], c3[:, t])
                    cb = stage2.tile([P, DQ], BF16, tag="castb")
                    eng = nc.gpsimd if t % 2 == 0 else nc.vector
                    eng.tensor_copy(cb[:, :DC], cf[:, :DC])
                    for ko in range(KC):
                        nc.sync.dma_start_transpose(
                            cT[:, ko, t * P:(t + 1) * P],
                            cb[:, ko * P:(ko + 1) * P])

                # ---------------- phase 2: projections ----------------
                # q^T[inner, q] and k^T[inner, kpos]
                for dst, w_b, src_t, nsub, nfree in (
                    (qT, wq_b, xT, KQ, NT), (kT, wk_b, cT, KC, NK),
                ):
                    for ko in range(KI):
                        for n0 in range(0, nfree, 512):
                            ps = psA.tile([P, 1024], F32, tag="big")
                            for kc in range(nsub):
                                nc.tensor.matmul(
                                    ps[:, 0:512],
                                    w_b[:, kc, ko * P:(ko + 1) * P],
                                    src_t[:, kc, n0:n0 + 512],
                                    start=(kc == 0), stop=(kc == nsub - 1))
                            nc.vector.tensor_copy(dst[:, ko, n0:n0 + 512],
                                                  ps[:, 0:512])
                # v[kpos, inner] into augmented layout
                for mt in range(KB):
                    for n0 in range(0, INNER, 512):
                        ps = psA.tile([P, 1024], F32, tag="big")
                        for kc in range(KC):
                            nc.tensor.matmul(
                                ps[:, 0:512],
                                cT[:, kc, mt * P:(mt + 1) * P],
                                wv_b[:, kc, n0:n0 + 512],
                                start=(kc == 0), stop=(kc == KC - 1))
                        h0 = n0 // DH
                        nc.vector.tensor_copy(
                            vA[:, mt, h0:h0 + 8, 0:DH],
                            ps[:, 0:512].rearrange("p (h d) -> p h d", d=DH))

            # ---------------- phase 3: attention per head ----------------
            expp = tc.alloc_tile_pool(name="expp", bufs=18)
            recp = tc.alloc_tile_pool(name="recp", bufs=2)
            for h in range(H):
                hp = (h % 2) * DH
                h2 = h // 2
                ets = []
                for kb in range(KB):
                    ps = psA.tile([P, 1024], F32, tag="big")
                    for n0 in (0, 512):
                        nc.tensor.matmul(
                            ps[:, n0:n0 + 512],
                            kT[hp:hp + DH, h2, kb * P:(kb + 1) * P],
                            qT[hp:hp + DH, h2, n0:n0 + 512],
                            start=True, stop=True)
                    et = expp.tile([P, NT], BF16, tag="exp")
                    nc.scalar.activation(et[:], ps[:], EXP, scale=float(SCALE))
                    ets.append(et)
                psv = psV.tile([DH + 1, NT], F32, tag="av")
                for n0 in (0, 512):
                    for kb in range(KB):
                        nc.tensor.matmul(
                            psv[:, n0:n0 + 512],
                            vA[:, kb, h, :],
                            ets[kb][:, n0:n0 + 512],
                            start=(kb == 0), stop=(kb == KB - 1))
                # normalize: attnT[head rows] = psv[0:64] * (1/psv[64])
                rec = recp.tile([1, NT], F32, tag="rec")
                nc.vector.reciprocal(rec[:], psv[DH:DH + 1, :])
                rb = recp.tile([DH, NT], F32, tag="rb")
                nc.gpsimd.partition_broadcast(rb[:], rec[0:1, :])
                nc.vector.tensor_tensor(attnT[hp:hp + DH, h2, :],
                                        psv[0:DH, :], rb[:], MULT)

            recp.release()
            expp.release()

            # ---------------- phase 4: out projection + bias ----------------
            outp = tc.alloc_tile_pool(name="outp", bufs=2)
            out3 = out_d.ap().rearrange("(t p) d -> p t d", p=P)
            for mt in range(TB):
                ps = psA.tile([P, 1024], F32, tag="big")
                for n0 in (0, 512):
                    for kc in range(KI):
                        nc.tensor.matmul(
                            ps[:, n0:n0 + 512],
                            attnT[:, kc, mt * P:(mt + 1) * P],
                            wo_b[:, kc, n0:n0 + 512],
                            start=(kc == 0), stop=False)
                    nc.tensor.matmul(
                        ps[:, n0:n0 + 512],
                        ones_b[0:1, :],
                        bo_b[0:1, n0:n0 + 512],
                        start=False, stop=True)
                ot = outp.tile([P, DQ], F32, tag="out")
                nc.vector.tensor_copy(ot[:], ps[:])
                nc.sync.dma_start(out3[:, mt], ot[:])
            outp.release()

    nc.compile()
    return nc


_NC_CACHE = None


def _make_in_maps(inputs):
    x = np.ascontiguousarray(np.asarray(inputs["x"], dtype=np.float32))
    context = np.ascontiguousarray(np.asarray(inputs["context"], dtype=np.float32))
    shared = {
        "wq": np.ascontiguousarray(np.asarray(inputs["Wq"], dtype=np.float32)),
        "wk": np.ascontiguousarray(np.asarray(inputs["Wk"], dtype=np.float32)),
        "wv": np.ascontiguousarray(np.asarray(inputs["Wv"], dtype=np.float32)),
        "wo": np.ascontiguousarray(np.asarray(inputs["Wo"], dtype=np.float32)),
        "bo": np.ascontiguousarray(np.asarray(inputs["bo"], dtype=np.float32)),
    }
    in_maps = []
    for c in range(N_CORES):
        b, s = divmod(c, 2)
        in_maps.append({
            "x": np.ascontiguousarray(x[b, s * NT:(s + 1) * NT, :]),
            "ctx": np.ascontiguousarray(context[b]),
            **shared,
        })
    return in_maps


def kernel(x, context, Wq, Wk, Wv, Wo, bo):
    global _NC_CACHE
    if _NC_CACHE is None:
        _NC_CACHE = build()
    nc = _NC_CACHE

    in_maps = _make_in_maps(dict(x=x, context=context, Wq=Wq, Wk=Wk, Wv=Wv,
                                 Wo=Wo, bo=bo))
    res = run_bass_kernel_spmd(nc, in_maps, core_ids=list(range(N_CORES)))
    out = np.empty((B, NQ_FULL, DQ), dtype=np.float32)
    for c in range(N_CORES):
        b, s = divmod(c, 2)
        out[b, s * NT:(s + 1) * NT, :] = res.results[c]["out"]
    return out


# revision 23
# speedup vs baseline: 1.3852x; 1.3852x over previous
"""CrossAttention kernel for 8 TRN2 NeuronCores.

Sharding: 8 cores = 4 batches x 2 query-halves (zero communication).
Each core computes all 16 heads for its 1024 queries:
  q^T = Wq^T x^T, k^T = Wk^T ctx^T, v = ctx Wv          (bf16 matmuls)
  scores^T[kpos, q] = k^T.T q^T / 8                      (K=64 per head,
      even/odd head pairs issued adjacently -> PE row-group concurrency)
  exp on ScalarE straight from PSUM (no max subtraction; scores ~ N(0,1))
  attn_out^T[d, q] + denominators via ones-augmented V (M=65 matmuls)
  batched reciprocal of all 16 head denominators, per-head GPSIMD
  partition-broadcast, out-proj with bias as a K=1 accumulating matmul.
"""

import sys

for _p in ("/opt/trn_rl_repo", "/root/.axon_site/_ro/trn_rl_repo"):
    if _p not in sys.path:
        sys.path.append(_p)

import numpy as np

import concourse.bass as bass
import concourse.tile as tile
from concourse import bacc, mybir
from concourse.bass_utils import run_bass_kernel_spmd

F32 = mybir.dt.float32
BF16 = mybir.dt.bfloat16
EXP = mybir.ActivationFunctionType.Exp
MULT = mybir.AluOpType.mult

P = 128
B, NQ_FULL, DQ = 4, 2048, 1024
NK, DC = 1024, 768
H, DH = 16, 64
INNER = H * DH  # 1024
NT = 1024  # local queries per core
N_CORES = 8

KQ = DQ // P      # 8
KC = DC // P      # 6
KI = INNER // P   # 8
TB = NT // P      # 8
KB = NK // P      # 8
SCALE = 1.0 / np.sqrt(DH)


def build(dbg=False):
    nc = bacc.Bacc("TRN2", target_bir_lowering=False, debug=False,
                   enable_asserts=False, num_devices=N_CORES)

    x_d = nc.dram_tensor("x", [NT, DQ], F32, kind="ExternalInput")
    ctx_d = nc.dram_tensor("ctx", [NK, DC], F32, kind="ExternalInput")
    wq_d = nc.dram_tensor("wq", [DQ, INNER], F32, kind="ExternalInput")
    wk_d = nc.dram_tensor("wk", [DC, INNER], F32, kind="ExternalInput")
    wv_d = nc.dram_tensor("wv", [DC, INNER], F32, kind="ExternalInput")
    wo_d = nc.dram_tensor("wo", [INNER, DQ], F32, kind="ExternalInput")
    bo_d = nc.dram_tensor("bo", [DQ], F32, kind="ExternalInput")
    out_d = nc.dram_tensor("out", [NT, DQ], F32, kind="ExternalOutput")
    if dbg:
        dqT = nc.dram_tensor("dqT", [P, KI, NT], F32, kind="ExternalOutput")
        dkT = nc.dram_tensor("dkT", [P, KI, NK], F32, kind="ExternalOutput")
        dvA = nc.dram_tensor("dvA", [P, KB, H, DH + 1], F32, kind="ExternalOutput")
        dav = nc.dram_tensor("dav", [P, H // 2, NT], F32, kind="ExternalOutput")
        dsums = nc.dram_tensor("dsums", [H, NT], F32, kind="ExternalOutput")
        drec = nc.dram_tensor("drec", [H, NT], F32, kind="ExternalOutput")
        dattnT = nc.dram_tensor("dattnT", [P, KI, NT], F32, kind="ExternalOutput")

    dmae = [nc.sync, nc.scalar]  # HWDGE dispatchers, round-robined

    with tile.TileContext(nc) as tc:
        with (
            tc.tile_pool(name="persist", bufs=1) as persist,
            tc.tile_pool(name="psA", bufs=2, space="PSUM") as psA,
            tc.tile_pool(name="psV", bufs=2, space="PSUM") as psV,
        ):
            qT = persist.tile([P, KI, NT], BF16)     # [inner, q]
            kT = persist.tile([P, KI, NK], BF16)     # [inner, kpos]
            vA = persist.tile([P, KB, H, DH + 1], BF16)  # [kpos, (head, d|1)]
            attnT = persist.tile([P, KI, NT], BF16)  # [inner, q]
            wo_b = persist.tile([P, KI, DQ], BF16)
            bo_b = persist.tile([1, DQ], BF16)
            ones_b = persist.tile([1, P], BF16)
            nc.vector.memset(ones_b[:], 1.0)
            nc.vector.memset(vA[:, :, :, DH:DH + 1], 1.0)

            dmai = 0

            def dma(out, in_):
                nonlocal dmai
                dmae[dmai % 2].dma_start(out, in_)
                dmai += 1

            def dmaT(out, in_):
                nc.sync.dma_start_transpose(out, in_)

            with tc.tile_pool(name="stage", bufs=1) as stage, \
                 tc.tile_pool(name="stage2", bufs=3) as stage2:
                xT = stage.tile([P, KQ, NT], BF16, tag="xT")
                cT = stage.tile([P, KC, NK], BF16, tag="cT")
                wq_b = stage.tile([P, KQ, INNER], BF16, tag="wq")
                wk_b = stage.tile([P, KC, INNER], BF16, tag="wk")
                wv_b = stage.tile([P, KC, INNER], BF16, tag="wv")

                # x: load natural (2 token-blocks at a time), cast, big transpose
                x4 = x_d.ap().rearrange("(t p) d -> p t d", p=P)
                for t2 in range(0, TB, 2):
                    xf = stage2.tile([P, 2, DQ], F32, tag="ldf")
                    dma(xf[:], x4[:, t2:t2 + 2])
                    xb = stage2.tile([P, 2, DQ], BF16, tag="castb")
                    nc.vector.tensor_copy(xb[:], xf[:])
                    for t in (t2, t2 + 1):
                        dmaT(xT[:, :, t * P:(t + 1) * P], xb[:, t - t2])

                # wq
                wq4 = wq_d.ap().rearrange("(o p) m -> p o m", p=P)
                for ko in range(0, KQ, 2):
                    wf = stage2.tile([P, 2, INNER], F32, tag="ldf")
                    dma(wf[:], wq4[:, ko:ko + 2])
                    nc.vector.tensor_copy(wq_b[:, ko:ko + 2], wf[:])

                # ---- q projection ----
                for ko in range(KI):
                    for n0 in range(0, NT, 512):
                        ps = psA.tile([P, 1024], F32, tag="big")
                        for kc in range(KQ):
                            nc.tensor.matmul(
                                ps[:, 0:512],
                                wq_b[:, kc, ko * P:(ko + 1) * P],
                                xT[:, kc, n0:n0 + 512],
                                start=(kc == 0), stop=(kc == KQ - 1))
                        nc.vector.tensor_copy(qT[:, ko, n0:n0 + 512],
                                              ps[:, 0:512])

                # ctx / wk / wv / wo / bo loads
                c4 = ctx_d.ap().rearrange("(t p) d -> p t d", p=P)
                for t2 in range(0, KB, 2):
                    cf = stage2.tile([P, 2, DQ], F32, tag="ldf")
                    dma(cf[:, :, :DC], c4[:, t2:t2 + 2])
                    cb = stage2.tile([P, 2, DQ], BF16, tag="castb")
                    nc.vector.tensor_copy(cb[:, :, :DC], cf[:, :, :DC])
                    for t in (t2, t2 + 1):
                        dmaT(cT[:, :, t * P:(t + 1) * P], cb[:, t - t2, :DC])
                wk4 = wk_d.ap().rearrange("(o p) m -> p o m", p=P)
                wv4 = wv_d.ap().rearrange("(o p) m -> p o m", p=P)
                for ko in range(0, KC, 2):
                    wf = stage2.tile([P, 2, INNER], F32, tag="ldf")
                    dma(wf[:], wk4[:, ko:ko + 2])
                    nc.vector.tensor_copy(wk_b[:, ko:ko + 2], wf[:])
                    wf2 = stage2.tile([P, 2, INNER], F32, tag="ldf")
                    dma(wf2[:], wv4[:, ko:ko + 2])
                    nc.vector.tensor_copy(wv_b[:, ko:ko + 2], wf2[:])
                wo4 = wo_d.ap().rearrange("(o p) m -> p o m", p=P)
                for ko in range(0, KI, 2):
                    wf = stage2.tile([P, 2, INNER], F32, tag="ldf")
                    dma(wf[:], wo4[:, ko:ko + 2])
                    nc.vector.tensor_copy(wo_b[:, ko:ko + 2], wf[:])
                bo_f = stage.tile([1, DQ], F32, tag="bo")
                dma(bo_f[:], bo_d.ap()[None, :])
                nc.vector.tensor_copy(bo_b[:], bo_f[:])

                # ---- k / v projections ----
                for ko in range(KI):
                    for n0 in range(0, NK, 512):
                        ps = psA.tile([P, 1024], F32, tag="big")
                        for kc in range(KC):
                            nc.tensor.matmul(
                                ps[:, 0:512],
                                wk_b[:, kc, ko * P:(ko + 1) * P],
                                cT[:, kc, n0:n0 + 512],
                                start=(kc == 0), stop=(kc == KC - 1))
                        nc.vector.tensor_copy(kT[:, ko, n0:n0 + 512],
                                              ps[:, 0:512])
                for mt in range(KB):
                    for n0 in range(0, INNER, 512):
                        ps = psA.tile([P, 1024], F32, tag="big")
                        for kc in range(KC):
                            nc.tensor.matmul(
                                ps[:, 0:512],
                                cT[:, kc, mt * P:(mt + 1) * P],
                                wv_b[:, kc, n0:n0 + 512],
                                start=(kc == 0), stop=(kc == KC - 1))
                        h0 = n0 // DH
                        nc.vector.tensor_copy(
                            vA[:, mt, h0:h0 + 8, 0:DH],
                            ps[:, 0:512].rearrange("p (h d) -> p h d", d=DH))

            # ---------------- attention, head pairs ----------------
            expp = tc.alloc_tile_pool(name="expp", bufs=8)
            avp = tc.alloc_tile_pool(name="avp", bufs=1)
            dramp = tc.alloc_tile_pool(name="dramp", bufs=1, space="DRAM")
            sums_dram = dramp.tile([H, NT], F32, name="sums_dram")
            av_sb = avp.tile([P, H // 2, NT], BF16, tag="avsb")  # [2*64d, hpair, q]
            for hp in range(H // 2):
                h0, h1 = 2 * hp, 2 * hp + 1
                h2 = hp
                psvs = {h: psV.tile([DH + 1, NT], F32, tag="av", name=f"psv{h}")
                        for h in (h0, h1)}
                for kb in range(KB):
                    pss = {}
                    for h in (h0, h1):
                        base = (h % 2) * DH
                        ps = psA.tile([P, 1024], F32, tag="big")
                        pss[h] = ps
                        for n0 in (0, 512):
                            nc.tensor.matmul(
                                ps[:, n0:n0 + 512],
                                kT[base:base + DH, h2, kb * P:(kb + 1) * P],
                                qT[base:base + DH, h2, n0:n0 + 512],
                                start=True, stop=True)
                    ets = {}
                    for h in (h0, h1):
                        et = expp.tile([P, NT], BF16, tag="exp")
                        nc.scalar.activation(et[:], pss[h][:], EXP,
                                             scale=float(SCALE))
                        ets[h] = et
                    for h in (h0, h1):
                        for n0 in (0, 512):
                            nc.tensor.matmul(
                                psvs[h][:, n0:n0 + 512],
                                vA[:, kb, h, :],
                                ets[h][:, n0:n0 + 512],
                                start=(kb == 0), stop=(kb == KB - 1))
                for i, h in enumerate((h0, h1)):
                    srow = expp.tile([1, NT], F32, tag="srow", name=f"srow{h}")
                    nc.vector.tensor_copy(srow[:], psvs[h][DH:DH + 1, :])
                    dma(sums_dram[h:h + 1, :], srow[:])
                    nc.vector.tensor_copy(av_sb[i * DH:(i + 1) * DH, hp, :],
                                          psvs[h][0:DH, :])

            # batched reciprocal of all denominators, then normalize
            sums16 = avp.tile([H, NT], F32, tag="sums")
            dma(sums16[:], sums_dram[:])
            rec16 = avp.tile([H, NT], F32, tag="rec")
            nc.vector.reciprocal(rec16[:], sums16[:])
            recp = tc.alloc_tile_pool(name="recp", bufs=4)
            rec_dram = dramp.tile([H, NT], F32, name="rec_dram")
            dma(rec_dram[:], rec16[:])
            for hp in range(H // 2):
                rb = recp.tile([P, NT], F32, tag="rb")
                for i in (0, 1):
                    src = rec_dram[2 * hp + i:2 * hp + i + 1, :]
                    bsrc = bass.AP(tensor=src.tensor, offset=src.offset,
                                   ap=[[0, DH]] + list(src.ap[1:]))
                    dma(rb[i * DH:(i + 1) * DH, :], bsrc)
                nc.vector.tensor_tensor(attnT[:, hp, :],
                                        av_sb[:, hp, :],
                                        rb[:], MULT)
            def dbg_dump(nc, tc, dma):
                dbgp = tc.alloc_tile_pool(name="dbgp", bufs=2)
                for name, t_sb, t_d in ((
                    "qT", qT, dqT), ("kT", kT, dkT), ("attnT", attnT, dattnT)):
                    for ko in range(KI):
                        f = dbgp.tile([P, NT], F32, tag="dbgf", name=f"dbg_{name}{ko}")
                        nc.vector.tensor_copy(f[:], t_sb[:, ko])
                        dma(t_d.ap().rearrange("p k n -> p k n")[:, ko], f[:])
                for kb in range(KB):
                    f = dbgp.tile([P, H * (DH + 1)], F32, tag="dbgf", name=f"dbg_v{kb}")
                    nc.vector.tensor_copy(f[:].rearrange("p (h d) -> p h d", d=DH+1), vA[:, kb])
                    dma(dvA.ap()[:, kb], f[:].rearrange("p (h d) -> p h d", d=DH+1))
                for hp2 in range(H // 2):
                    f = dbgp.tile([P, NT], F32, tag="dbgf", name=f"dbg_av{hp2}")
                    nc.vector.tensor_copy(f[:], av_sb2[:, hp2])
                    dma(dav.ap()[:, hp2], f[:])
                dma(dsums.ap(), sums16[:])
                dma(drec.ap(), rec16[:])
                dbgp.release()


            av_sb2 = av_sb
            if dbg:
                dbg_dump(nc, tc, dma)
            recp.release()
            avp.release()
            expp.release()
            dramp.release()

            # ---------------- out projection + bias ----------------
            outp = tc.alloc_tile_pool(name="outp", bufs=2)
            out3 = out_d.ap().rearrange("(t p) d -> p t d", p=P)
            for mt in range(TB):
                ps = psA.tile([P, 1024], F32, tag="big")
                for n0 in (0, 512):
                    for kc in range(KI):
                        nc.tensor.matmul(
                            ps[:, n0:n0 + 512],
                            attnT[:, kc, mt * P:(mt + 1) * P],
                            wo_b[:, kc, n0:n0 + 512],
                            start=(kc == 0), stop=False)
                    nc.tensor.matmul(
                        ps[:, n0:n0 + 512],
                        ones_b[0:1, :],
                        bo_b[0:1, n0:n0 + 512],
                        start=False, stop=True)
                ot = outp.tile([P, DQ], F32, tag="out")
                nc.vector.tensor_copy(ot[:], ps[:])
                dma(out3[:, mt], ot[:])
            outp.release()

    nc.compile()
    return nc


_NC_CACHE = None


def _make_in_maps(inputs):
    x = np.ascontiguousarray(np.asarray(inputs["x"], dtype=np.float32))
    context = np.ascontiguousarray(np.asarray(inputs["context"], dtype=np.float32))
    shared = {
        "wq": np.ascontiguousarray(np.asarray(inputs["Wq"], dtype=np.float32)),
        "wk": np.ascontiguousarray(np.asarray(inputs["Wk"], dtype=np.float32)),
        "wv": np.ascontiguousarray(np.asarray(inputs["Wv"], dtype=np.float32)),
        "wo": np.ascontiguousarray(np.asarray(inputs["Wo"], dtype=np.float32)),
        "bo": np.ascontiguousarray(np.asarray(inputs["bo"], dtype=np.float32)),
    }
    in_maps = []
    for c in range(N_CORES):
        b, s = divmod(c, 2)
        in_maps.append({
            "x": np.ascontiguousarray(x[b, s * NT:(s + 1) * NT, :]),
            "ctx": np.ascontiguousarray(context[b]),
            **shared,
        })
    return in_maps


def kernel(x, context, Wq, Wk, Wv, Wo, bo):
    global _NC_CACHE
    if _NC_CACHE is None:
        _NC_CACHE = build()
    nc = _NC_CACHE

    in_maps = _make_in_maps(dict(x=x, context=context, Wq=Wq, Wk=Wk, Wv=Wv,
                                 Wo=Wo, bo=bo))
    res = run_bass_kernel_spmd(nc, in_maps, core_ids=list(range(N_CORES)))
    out = np.empty((B, NQ_FULL, DQ), dtype=np.float32)
    for c in range(N_CORES):
        b, s = divmod(c, 2)
        out[b, s * NT:(s + 1) * NT, :] = res.results[c]["out"]
    return out


# revision 25
# speedup vs baseline: 1.4897x; 1.0754x over previous
"""CrossAttention kernel for 8 TRN2 NeuronCores.

Sharding: 8 cores = 4 batches x 2 query-halves (zero communication).
Each core computes all 16 heads for its 1024 queries:
  q^T = Wq^T x^T, k^T = Wk^T ctx^T, v = ctx Wv          (bf16 matmuls)
  scores^T[kpos, q] = k^T.T q^T / 8                      (K=64 per head,
      even/odd head pairs issued adjacently -> PE row-group concurrency)
  exp on ScalarE straight from PSUM (no max subtraction; scores ~ N(0,1))
  attn_out^T[d, q] + denominators via ones-augmented V (M=65 matmuls)
  batched reciprocal of all 16 head denominators, per-head GPSIMD
  partition-broadcast, out-proj with bias as a K=1 accumulating matmul.
"""

import sys

for _p in ("/opt/trn_rl_repo", "/root/.axon_site/_ro/trn_rl_repo"):
    if _p not in sys.path:
        sys.path.append(_p)

import numpy as np

import concourse.bass as bass
import concourse.tile as tile
from concourse import bacc, mybir
from concourse.bass_utils import run_bass_kernel_spmd

F32 = mybir.dt.float32
BF16 = mybir.dt.bfloat16
EXP = mybir.ActivationFunctionType.Exp
MULT = mybir.AluOpType.mult

P = 128
B, NQ_FULL, DQ = 4, 2048, 1024
NK, DC = 1024, 768
H, DH = 16, 64
INNER = H * DH  # 1024
NT = 1024  # local queries per core
N_CORES = 8

KQ = DQ // P      # 8
KC = DC // P      # 6
KI = INNER // P   # 8
TB = NT // P      # 8
KB = NK // P      # 8
SCALE = 1.0 / np.sqrt(DH)


def build(dbg=False):
    nc = bacc.Bacc("TRN2", target_bir_lowering=False, debug=False,
                   enable_asserts=False, num_devices=N_CORES)

    x_d = nc.dram_tensor("x", [NT, DQ], F32, kind="ExternalInput")
    ctx_d = nc.dram_tensor("ctx", [NK, DC], F32, kind="ExternalInput")
    wq_d = nc.dram_tensor("wq", [DQ, INNER], F32, kind="ExternalInput")
    wk_d = nc.dram_tensor("wk", [DC, INNER], F32, kind="ExternalInput")
    wv_d = nc.dram_tensor("wv", [DC, INNER], F32, kind="ExternalInput")
    wo_d = nc.dram_tensor("wo", [INNER, DQ], F32, kind="ExternalInput")
    bo_d = nc.dram_tensor("bo", [DQ], F32, kind="ExternalInput")
    out_d = nc.dram_tensor("out", [NT, DQ], F32, kind="ExternalOutput")
    if dbg:
        dqT = nc.dram_tensor("dqT", [P, KI, NT], F32, kind="ExternalOutput")
        dkT = nc.dram_tensor("dkT", [P, KI, NK], F32, kind="ExternalOutput")
        dvA = nc.dram_tensor("dvA", [P, KB, H, DH + 1], F32, kind="ExternalOutput")
        dav = nc.dram_tensor("dav", [P, H // 2, NT], F32, kind="ExternalOutput")
        dsums = nc.dram_tensor("dsums", [H, NT], F32, kind="ExternalOutput")
        drec = nc.dram_tensor("drec", [H, NT], F32, kind="ExternalOutput")
        dattnT = nc.dram_tensor("dattnT", [P, KI, NT], F32, kind="ExternalOutput")

    dmae = [nc.sync, nc.scalar]  # HWDGE dispatchers, round-robined

    with tile.TileContext(nc) as tc:
        with (
            tc.tile_pool(name="persist", bufs=1) as persist,
            tc.tile_pool(name="psA", bufs=3, space="PSUM") as psA,
            tc.tile_pool(name="psV", bufs=2, space="PSUM") as psV,
        ):
            qT = persist.tile([P, KI, NT], BF16)     # [inner, q]
            kT = persist.tile([P, KI, NK], BF16)     # [inner, kpos]
            vA = persist.tile([P, KB, H, DH + 1], BF16)  # [kpos, (head, d|1)]
            attnT = persist.tile([P, KI, NT], BF16)  # [inner, q]
            wo_b = persist.tile([P, KI, DQ], BF16)
            bo_b = persist.tile([1, DQ], BF16)
            ones_b = persist.tile([1, P], BF16)
            nc.vector.memset(ones_b[:], 1.0)
            nc.vector.memset(vA[:, :, :, DH:DH + 1], 1.0)

            dmai = 0

            def dma(out, in_):
                nonlocal dmai
                dmae[dmai % 2].dma_start(out, in_)
                dmai += 1

            def dmaT(out, in_):
                nc.sync.dma_start_transpose(out, in_)

            with tc.tile_pool(name="stage", bufs=1) as stage, \
                 tc.tile_pool(name="stage2", bufs=3) as stage2:
                xT = stage.tile([P, KQ, NT], BF16, tag="xT")
                cT = stage.tile([P, KC, NK], BF16, tag="cT")
                wq_b = stage.tile([P, KQ, INNER], BF16, tag="wq")
                wk_b = stage.tile([P, KC, INNER], BF16, tag="wk")
                wv_b = stage.tile([P, KC, INNER], BF16, tag="wv")

                # x: load natural (2 token-blocks at a time), cast, big transpose
                x4 = x_d.ap().rearrange("(t p) d -> p t d", p=P)
                for t2 in range(0, TB, 2):
                    xf = stage2.tile([P, 2, DQ], F32, tag="ldf")
                    dma(xf[:], x4[:, t2:t2 + 2])
                    xb = stage2.tile([P, 2, DQ], BF16, tag="castb")
                    nc.vector.tensor_copy(xb[:], xf[:])
                    for t in (t2, t2 + 1):
                        dmaT(xT[:, :, t * P:(t + 1) * P], xb[:, t - t2])

                # wq
                wq4 = wq_d.ap().rearrange("(o p) m -> p o m", p=P)
                for ko in range(0, KQ, 2):
                    wf = stage2.tile([P, 2, INNER], F32, tag="ldf")
                    dma(wf[:], wq4[:, ko:ko + 2])
                    nc.vector.tensor_copy(wq_b[:, ko:ko + 2], wf[:])

                # ---- q projection ----
                for ko in range(KI):
                    for n0 in range(0, NT, 512):
                        ps = psA.tile([P, 1024], F32, tag="big")
                        for kc in range(KQ):
                            nc.tensor.matmul(
                                ps[:, 0:512],
                                wq_b[:, kc, ko * P:(ko + 1) * P],
                                xT[:, kc, n0:n0 + 512],
                                start=(kc == 0), stop=(kc == KQ - 1))
                        nc.vector.tensor_copy(qT[:, ko, n0:n0 + 512],
                                              ps[:, 0:512])

                # ctx / wk / wv / wo / bo loads
                c4 = ctx_d.ap().rearrange("(t p) d -> p t d", p=P)
                for t2 in range(0, KB, 2):
                    cf = stage2.tile([P, 2, DQ], F32, tag="ldf")
                    dma(cf[:, :, :DC], c4[:, t2:t2 + 2])
                    cb = stage2.tile([P, 2, DQ], BF16, tag="castb")
                    nc.vector.tensor_copy(cb[:, :, :DC], cf[:, :, :DC])
                    for t in (t2, t2 + 1):
                        dmaT(cT[:, :, t * P:(t + 1) * P], cb[:, t - t2, :DC])
                wk4 = wk_d.ap().rearrange("(o p) m -> p o m", p=P)
                wv4 = wv_d.ap().rearrange("(o p) m -> p o m", p=P)
                for ko in range(0, KC, 2):
                    wf = stage2.tile([P, 2, INNER], F32, tag="ldf")
                    dma(wf[:], wk4[:, ko:ko + 2])
                    nc.vector.tensor_copy(wk_b[:, ko:ko + 2], wf[:])
                    wf2 = stage2.tile([P, 2, INNER], F32, tag="ldf")
                    dma(wf2[:], wv4[:, ko:ko + 2])
                    nc.vector.tensor_copy(wv_b[:, ko:ko + 2], wf2[:])
                wo4 = wo_d.ap().rearrange("(o p) m -> p o m", p=P)
                for ko in range(0, KI, 2):
                    wf = stage2.tile([P, 2, INNER], F32, tag="ldf")
                    dma(wf[:], wo4[:, ko:ko + 2])
                    nc.vector.tensor_copy(wo_b[:, ko:ko + 2], wf[:])
                bo_f = stage.tile([1, DQ], F32, tag="bo")
                dma(bo_f[:], bo_d.ap()[None, :])
                nc.vector.tensor_copy(bo_b[:], bo_f[:])

                # ---- k / v projections ----
                for ko in range(KI):
                    for n0 in range(0, NK, 512):
                        ps = psA.tile([P, 1024], F32, tag="big")
                        for kc in range(KC):
                            nc.tensor.matmul(
                                ps[:, 0:512],
                                wk_b[:, kc, ko * P:(ko + 1) * P],
                                cT[:, kc, n0:n0 + 512],
                                start=(kc == 0), stop=(kc == KC - 1))
                        nc.vector.tensor_copy(kT[:, ko, n0:n0 + 512],
                                              ps[:, 0:512])
                for mt in range(KB):
                    for n0 in range(0, INNER, 512):
                        ps = psA.tile([P, 1024], F32, tag="big")
                        for kc in range(KC):
                            nc.tensor.matmul(
                                ps[:, 0:512],
                                cT[:, kc, mt * P:(mt + 1) * P],
                                wv_b[:, kc, n0:n0 + 512],
                                start=(kc == 0), stop=(kc == KC - 1))
                        h0 = n0 // DH
                        nc.vector.tensor_copy(
                            vA[:, mt, h0:h0 + 8, 0:DH],
                            ps[:, 0:512].rearrange("p (h d) -> p h d", d=DH))

            # ---------------- attention, head pairs ----------------
            expp = tc.alloc_tile_pool(name="expp", bufs=12)
            avp = tc.alloc_tile_pool(name="avp", bufs=1)
            dramp = tc.alloc_tile_pool(name="dramp", bufs=1, space="DRAM")
            sums_dram = dramp.tile([H, NT], F32, name="sums_dram")
            av_sb = avp.tile([P, H // 2, NT], BF16, tag="avsb")  # [2*64d, hpair, q]
            def attn_head_pair(hp):
                h0, h1 = 2 * hp, 2 * hp + 1
                h2 = hp
                psvs = {h: [psV.tile([DH + 1, 512], F32, tag="av",
                                     name=f"psv{h}_{n}") for n in (0, 1)]
                        for h in (h0, h1)}
                ets_all = {h0: [], h1: []}
                for kb in range(KB):
                    pss = {}
                    for h in (h0, h1):
                        base = (h % 2) * DH
                        ps = psA.tile([P, 1024], F32, tag="big")
                        pss[h] = ps
                        for n0 in (0, 512):
                            nc.tensor.matmul(
                                ps[:, n0:n0 + 512],
                                kT[base:base + DH, h2, kb * P:(kb + 1) * P],
                                qT[base:base + DH, h2, n0:n0 + 512],
                                start=True, stop=True)
                    for h in (h0, h1):
                        et = expp.tile([P, NT], BF16, tag="exp")
                        nc.scalar.activation(et[:], pss[h][:], EXP,
                                             scale=float(SCALE))
                        ets_all[h].append(et)
                    for h in (h0, h1):
                        for ni, n0 in enumerate((0, 512)):
                            nc.tensor.matmul(
                                psvs[h][ni][:],
                                vA[:, kb, h, :],
                                ets_all[h][kb][:, n0:n0 + 512],
                                start=(kb == 0), stop=(kb == KB - 1))
                for i, h in enumerate((h0, h1)):
                    for ni, n0 in enumerate((0, 512)):
                        srow = expp.tile([1, 512], F32, tag="srow",
                                         name=f"srow{h}_{ni}")
                        nc.vector.tensor_copy(srow[:], psvs[h][ni][DH:DH + 1, :])
                        dma(sums_dram[h:h + 1, n0:n0 + 512], srow[:])
                        nc.vector.tensor_copy(
                            av_sb[i * DH:(i + 1) * DH, hp, n0:n0 + 512],
                            psvs[h][ni][0:DH, :])

            def normalize_batch(bi):
                sums8 = avp.tile([H // 2, NT], F32, tag=f"sums{bi}",
                                 name=f"sums_b{bi}")
                dma(sums8[:], sums_dram[bi * 8:(bi + 1) * 8, :])
                rec8 = avp.tile([H // 2, NT], F32, tag=f"rec{bi}",
                                name=f"rec_b{bi}")
                nc.vector.reciprocal(rec8[:], sums8[:])
                dma(rec_dram[bi * 8:(bi + 1) * 8, :], rec8[:])
                for hp in range(bi * 4, (bi + 1) * 4):
                    rb = recp.tile([P, NT], F32, tag="rb")
                    for i in (0, 1):
                        src = rec_dram[2 * hp + i:2 * hp + i + 1, :]
                        bsrc = bass.AP(tensor=src.tensor, offset=src.offset,
                                       ap=[[0, DH]] + list(src.ap[1:]))
                        dma(rb[i * DH:(i + 1) * DH, :], bsrc)
                    nc.vector.tensor_tensor(attnT[:, hp, :],
                                            av_sb[:, hp, :],
                                            rb[:], MULT)

            recp = tc.alloc_tile_pool(name="recp", bufs=4)
            rec_dram = dramp.tile([H, NT], F32, name="rec_dram")
            for hp in range(H // 2):
                attn_head_pair(hp)
                if hp == 3:
                    normalize_batch(0)
            normalize_batch(1)

            def dbg_dump(nc, tc, dma):
                dbgp = tc.alloc_tile_pool(name="dbgp", bufs=2)
                for name, t_sb, t_d in ((
                    "qT", qT, dqT), ("kT", kT, dkT), ("attnT", attnT, dattnT)):
                    for ko in range(KI):
                        f = dbgp.tile([P, NT], F32, tag="dbgf", name=f"dbg_{name}{ko}")
                        nc.vector.tensor_copy(f[:], t_sb[:, ko])
                        dma(t_d.ap().rearrange("p k n -> p k n")[:, ko], f[:])
                for kb in range(KB):
                    f = dbgp.tile([P, H * (DH + 1)], F32, tag="dbgf", name=f"dbg_v{kb}")
                    nc.vector.tensor_copy(f[:].rearrange("p (h d) -> p h d", d=DH+1), vA[:, kb])
                    dma(dvA.ap()[:, kb], f[:].rearrange("p (h d) -> p h d", d=DH+1))
                for hp2 in range(H // 2):
                    f = dbgp.tile([P, NT], F32, tag="dbgf", name=f"dbg_av{hp2}")
                    nc.vector.tensor_copy(f[:], av_sb2[:, hp2])
                    dma(dav.ap()[:, hp2], f[:])
                dma(dsums.ap(), sums16[:])
                dma(drec.ap(), rec16[:])
                dbgp.release()


            av_sb2 = av_sb
            if dbg:
                dbg_dump(nc, tc, dma)
            recp.release()
            avp.release()
            expp.release()
            dramp.release()

            # ---------------- out projection + bias ----------------
            outp = tc.alloc_tile_pool(name="outp", bufs=2)
            out3 = out_d.ap().rearrange("(t p) d -> p t d", p=P)
            for mt in range(TB):
                ps = psA.tile([P, 1024], F32, tag="big")
                for n0 in (0, 512):
                    for kc in range(KI):
                        nc.tensor.matmul(
                            ps[:, n0:n0 + 512],
                            attnT[:, kc, mt * P:(mt + 1) * P],
                            wo_b[:, kc, n0:n0 + 512],
                            start=(kc == 0), stop=False)
                    nc.tensor.matmul(
                        ps[:, n0:n0 + 512],
                        ones_b[0:1, :],
                        bo_b[0:1, n0:n0 + 512],
                        start=False, stop=True)
                ot = outp.tile([P, DQ], F32, tag="out")
                nc.vector.tensor_copy(ot[:], ps[:])
                dma(out3[:, mt], ot[:])
            outp.release()

    nc.compile()
    return nc


_NC_CACHE = None


def _make_in_maps(inputs):
    x = np.ascontiguousarray(np.asarray(inputs["x"], dtype=np.float32))
    context = np.ascontiguousarray(np.asarray(inputs["context"], dtype=np.float32))
    shared = {
        "wq": np.ascontiguousarray(np.asarray(inputs["Wq"], dtype=np.float32)),
        "wk": np.ascontiguousarray(np.asarray(inputs["Wk"], dtype=np.float32)),
        "wv": np.ascontiguousarray(np.asarray(inputs["Wv"], dtype=np.float32)),
        "wo": np.ascontiguousarray(np.asarray(inputs["Wo"], dtype=np.float32)),
        "bo": np.ascontiguousarray(np.asarray(inputs["bo"], dtype=np.float32)),
    }
    in_maps = []
    for c in range(N_CORES):
        b, s = divmod(c, 2)
        in_maps.append({
            "x": np.ascontiguousarray(x[b, s * NT:(s + 1) * NT, :]),
            "ctx": np.ascontiguousarray(context[b]),
            **shared,
        })
    return in_maps


def kernel(x, context, Wq, Wk, Wv, Wo, bo):
    global _NC_CACHE
    if _NC_CACHE is None:
        _NC_CACHE = build()
    nc = _NC_CACHE

    in_maps = _make_in_maps(dict(x=x, context=context, Wq=Wq, Wk=Wk, Wv=Wv,
                                 Wo=Wo, bo=bo))
    res = run_bass_kernel_spmd(nc, in_maps, core_ids=list(range(N_CORES)))
    out = np.empty((B, NQ_FULL, DQ), dtype=np.float32)
    for c in range(N_CORES):
        b, s = divmod(c, 2)
        out[b, s * NT:(s + 1) * NT, :] = res.results[c]["out"]
    return out


# revision 27
# speedup vs baseline: 1.5527x; 1.0423x over previous
"""CrossAttention kernel for 8 TRN2 NeuronCores.

Sharding: 8 cores = 4 batches x 2 query-halves (zero communication).
Each core computes all 16 heads for its 1024 queries:
  q^T = Wq^T x^T, k^T = Wk^T ctx^T, v = ctx Wv          (bf16 matmuls)
  scores^T[kpos, q] = k^T.T q^T / 8                      (K=64 per head,
      even/odd head pairs issued adjacently -> PE row-group concurrency)
  exp on ScalarE straight from PSUM (no max subtraction; scores ~ N(0,1))
  attn_out^T[d, q] + denominators via ones-augmented V (M=65 matmuls)
  batched reciprocal of all 16 head denominators, per-head GPSIMD
  partition-broadcast, out-proj with bias as a K=1 accumulating matmul.
"""

import sys

for _p in ("/opt/trn_rl_repo", "/root/.axon_site/_ro/trn_rl_repo"):
    if _p not in sys.path:
        sys.path.append(_p)

import numpy as np

import concourse.bass as bass
import concourse.tile as tile
from concourse import bacc, mybir
from concourse.bass_utils import run_bass_kernel_spmd

F32 = mybir.dt.float32
BF16 = mybir.dt.bfloat16
EXP = mybir.ActivationFunctionType.Exp
MULT = mybir.AluOpType.mult

P = 128
B, NQ_FULL, DQ = 4, 2048, 1024
NK, DC = 1024, 768
H, DH = 16, 64
INNER = H * DH  # 1024
NT = 1024  # local queries per core
N_CORES = 8

KQ = DQ // P      # 8
KC = DC // P      # 6
KI = INNER // P   # 8
TB = NT // P      # 8
KB = NK // P      # 8
SCALE = 1.0 / np.sqrt(DH)


def build(dbg=False):
    nc = bacc.Bacc("TRN2", target_bir_lowering=False, debug=False,
                   enable_asserts=False, num_devices=N_CORES)

    x_d = nc.dram_tensor("x", [NT, DQ], F32, kind="ExternalInput")
    ctx_d = nc.dram_tensor("ctx", [NK, DC], F32, kind="ExternalInput")
    wq_d = nc.dram_tensor("wq", [DQ, INNER], F32, kind="ExternalInput")
    wk_d = nc.dram_tensor("wk", [DC, INNER], F32, kind="ExternalInput")
    wv_d = nc.dram_tensor("wv", [DC, INNER], F32, kind="ExternalInput")
    wo_d = nc.dram_tensor("wo", [INNER, DQ], F32, kind="ExternalInput")
    bo_d = nc.dram_tensor("bo", [DQ], F32, kind="ExternalInput")
    out_d = nc.dram_tensor("out", [NT, DQ], F32, kind="ExternalOutput")
    if dbg:
        dqT = nc.dram_tensor("dqT", [P, KI, NT], F32, kind="ExternalOutput")
        dkT = nc.dram_tensor("dkT", [P, KI, NK], F32, kind="ExternalOutput")
        dvA = nc.dram_tensor("dvA", [P, KB, H, DH + 1], F32, kind="ExternalOutput")
        dav = nc.dram_tensor("dav", [P, H // 2, NT], F32, kind="ExternalOutput")
        dsums = nc.dram_tensor("dsums", [H, NT], F32, kind="ExternalOutput")
        drec = nc.dram_tensor("drec", [H, NT], F32, kind="ExternalOutput")
        dattnT = nc.dram_tensor("dattnT", [P, KI, NT], F32, kind="ExternalOutput")

    dmae = [nc.sync, nc.scalar]  # HWDGE dispatchers, round-robined

    with tile.TileContext(nc) as tc:
        with (
            tc.tile_pool(name="persist", bufs=1) as persist,
            tc.tile_pool(name="psA", bufs=3, space="PSUM") as psA,
            tc.tile_pool(name="psV", bufs=2, space="PSUM") as psV,
        ):
            qT = persist.tile([P, KI, NT], BF16)     # [inner, q]
            kT = persist.tile([P, KI, NK], BF16)     # [inner, kpos]
            vA = persist.tile([P, KB, H, DH + 1], BF16)  # [kpos, (head, d|1)]
            attnT = persist.tile([P, KI, NT], BF16)  # [inner, q]
            wo_b = persist.tile([P, KI, DQ], BF16)
            bo_b = persist.tile([1, DQ], BF16)
            ones_b = persist.tile([1, P], BF16)
            nc.vector.memset(ones_b[:], 1.0)
            nc.vector.memset(vA[:, :, :, DH:DH + 1], 1.0)

            dmai = 0

            def dma(out, in_):
                nonlocal dmai
                dmae[dmai % 2].dma_start(out, in_)
                dmai += 1

            def dmaT(out, in_):
                nc.sync.dma_start_transpose(out, in_)

            with tc.tile_pool(name="stage", bufs=1) as stage, \
                 tc.tile_pool(name="stage2", bufs=3) as stage2:
                xT = stage.tile([P, KQ, NT], BF16, tag="xT")
                cT = stage.tile([P, KC, NK], BF16, tag="cT")
                wq_b = stage.tile([P, KQ, INNER], BF16, tag="wq")
                wk_b = stage.tile([P, KC, INNER], BF16, tag="wk")
                wv_b = stage.tile([P, KC, INNER], BF16, tag="wv")

                # x: load natural (2 token-blocks at a time), cast, big transpose
                x4 = x_d.ap().rearrange("(t p) d -> p t d", p=P)
                for t2 in range(0, TB, 2):
                    xf = stage2.tile([P, 2, DQ], F32, tag="ldf")
                    dma(xf[:], x4[:, t2:t2 + 2])
                    xb = stage2.tile([P, 2, DQ], BF16, tag="castb")
                    nc.vector.tensor_copy(xb[:], xf[:])
                    for t in (t2, t2 + 1):
                        dmaT(xT[:, :, t * P:(t + 1) * P], xb[:, t - t2])

                # wq
                wq4 = wq_d.ap().rearrange("(o p) m -> p o m", p=P)
                for ko in range(0, KQ, 2):
                    wf = stage2.tile([P, 2, INNER], F32, tag="ldf")
                    dma(wf[:], wq4[:, ko:ko + 2])
                    nc.scalar.copy(wq_b[:, ko:ko + 2], wf[:])

                # ---- q projection ----
                for ko in range(KI):
                    for n0 in range(0, NT, 512):
                        ps = psA.tile([P, 1024], F32, tag="big")
                        for kc in range(KQ):
                            nc.tensor.matmul(
                                ps[:, 0:512],
                                wq_b[:, kc, ko * P:(ko + 1) * P],
                                xT[:, kc, n0:n0 + 512],
                                start=(kc == 0), stop=(kc == KQ - 1))
                        nc.vector.tensor_copy(qT[:, ko, n0:n0 + 512],
                                              ps[:, 0:512])

                # ctx / wk / wv / wo / bo loads
                c4 = ctx_d.ap().rearrange("(t p) d -> p t d", p=P)
                for t2 in range(0, KB, 2):
                    cf = stage2.tile([P, 2, DQ], F32, tag="ldf")
                    dma(cf[:, :, :DC], c4[:, t2:t2 + 2])
                    cb = stage2.tile([P, 2, DQ], BF16, tag="castb")
                    nc.vector.tensor_copy(cb[:, :, :DC], cf[:, :, :DC])
                    for t in (t2, t2 + 1):
                        dmaT(cT[:, :, t * P:(t + 1) * P], cb[:, t - t2, :DC])
                wk4 = wk_d.ap().rearrange("(o p) m -> p o m", p=P)
                wv4 = wv_d.ap().rearrange("(o p) m -> p o m", p=P)
                for ko in range(0, KC, 2):
                    wf = stage2.tile([P, 2, INNER], F32, tag="ldf")
                    dma(wf[:], wk4[:, ko:ko + 2])
                    nc.scalar.copy(wk_b[:, ko:ko + 2], wf[:])
                    wf2 = stage2.tile([P, 2, INNER], F32, tag="ldf")
                    dma(wf2[:], wv4[:, ko:ko + 2])
                    nc.scalar.copy(wv_b[:, ko:ko + 2], wf2[:])
                wo4 = wo_d.ap().rearrange("(o p) m -> p o m", p=P)
                for ko in range(0, KI, 2):
                    wf = stage2.tile([P, 2, INNER], F32, tag="ldf")
                    dma(wf[:], wo4[:, ko:ko + 2])
                    nc.scalar.copy(wo_b[:, ko:ko + 2], wf[:])
                bo_f = stage.tile([1, DQ], F32, tag="bo")
                dma(bo_f[:], bo_d.ap()[None, :])
                nc.scalar.copy(bo_b[:], bo_f[:])

                # ---- k / v projections ----
                for ko in range(KI):
                    for n0 in range(0, NK, 512):
                        ps = psA.tile([P, 1024], F32, tag="big")
                        for kc in range(KC):
                            nc.tensor.matmul(
                                ps[:, 0:512],
                                wk_b[:, kc, ko * P:(ko + 1) * P],
                                cT[:, kc, n0:n0 + 512],
                                start=(kc == 0), stop=(kc == KC - 1))
                        nc.vector.tensor_copy(kT[:, ko, n0:n0 + 512],
                                              ps[:, 0:512])
                for mt in range(KB):
                    for n0 in range(0, INNER, 512):
                        ps = psA.tile([P, 1024], F32, tag="big")
                        for kc in range(KC):
                            nc.tensor.matmul(
                                ps[:, 0:512],
                                cT[:, kc, mt * P:(mt + 1) * P],
                                wv_b[:, kc, n0:n0 + 512],
                                start=(kc == 0), stop=(kc == KC - 1))
                        h0 = n0 // DH
                        nc.vector.tensor_copy(
                            vA[:, mt, h0:h0 + 8, 0:DH],
                            ps[:, 0:512].rearrange("p (h d) -> p h d", d=DH))

            # ---------------- attention, head pairs ----------------
            expp = tc.alloc_tile_pool(name="expp", bufs=12)
            avp = tc.alloc_tile_pool(name="avp", bufs=1)
            dramp = tc.alloc_tile_pool(name="dramp", bufs=1, space="DRAM")
            sums_dram = dramp.tile([H, NT], F32, name="sums_dram")
            av_sb = avp.tile([P, H // 2, NT], BF16, tag="avsb")  # [2*64d, hpair, q]
            def attn_head_pair(hp):
                h0, h1 = 2 * hp, 2 * hp + 1
                h2 = hp
                psvs = {h: [psV.tile([DH + 1, 512], F32, tag="av",
                                     name=f"psv{h}_{n}") for n in (0, 1)]
                        for h in (h0, h1)}
                ets_all = {h0: [], h1: []}
                for kb in range(KB):
                    pss = {}
                    for h in (h0, h1):
                        base = (h % 2) * DH
                        ps = psA.tile([P, 1024], F32, tag="big")
                        pss[h] = ps
                        for n0 in (0, 512):
                            nc.tensor.matmul(
                                ps[:, n0:n0 + 512],
                                kT[base:base + DH, h2, kb * P:(kb + 1) * P],
                                qT[base:base + DH, h2, n0:n0 + 512],
                                start=True, stop=True)
                    for h in (h0, h1):
                        et = expp.tile([P, NT], BF16, tag="exp")
                        nc.scalar.activation(et[:], pss[h][:], EXP,
                                             scale=float(SCALE))
                        ets_all[h].append(et)
                    for h in (h0, h1):
                        for ni, n0 in enumerate((0, 512)):
                            nc.tensor.matmul(
                                psvs[h][ni][:],
                                vA[:, kb, h, :],
                                ets_all[h][kb][:, n0:n0 + 512],
                                start=(kb == 0), stop=(kb == KB - 1))
                for i, h in enumerate((h0, h1)):
                    for ni, n0 in enumerate((0, 512)):
                        srow = expp.tile([1, 512], F32, tag="srow",
                                         name=f"srow{h}_{ni}")
                        nc.vector.tensor_copy(srow[:], psvs[h][ni][DH:DH + 1, :])
                        dma(sums_dram[h:h + 1, n0:n0 + 512], srow[:])
                        nc.vector.tensor_copy(
                            av_sb[i * DH:(i + 1) * DH, hp, n0:n0 + 512],
                            psvs[h][ni][0:DH, :])

            def normalize_batch(bi):
                sums8 = avp.tile([DH, P], F32, tag=f"sums{bi}",
                                 name=f"sums_b{bi}")
                dma(sums8[:], sums_dram[:]
                    .rearrange("h (a b) -> (h a) b", b=P)[bi * DH:(bi + 1) * DH])
                rec8 = avp.tile([DH, P], F32, tag=f"rec{bi}",
                                name=f"rec_b{bi}")
                nc.vector.reciprocal(rec8[:], sums8[:])
                dma(rec_dram[:]
                    .rearrange("h (a b) -> (h a) b", b=P)[bi * DH:(bi + 1) * DH],
                    rec8[:])
                for hp in range(bi * 4, (bi + 1) * 4):
                    rb = recp.tile([P, NT], F32, tag="rb")
                    for i in (0, 1):
                        src = rec_dram[2 * hp + i:2 * hp + i + 1, :]
                        bsrc = bass.AP(tensor=src.tensor, offset=src.offset,
                                       ap=[[0, DH]] + list(src.ap[1:]))
                        dma(rb[i * DH:(i + 1) * DH, :], bsrc)
                    nc.vector.tensor_tensor(attnT[:, hp, :],
                                            av_sb[:, hp, :],
                                            rb[:], MULT)

            recp = tc.alloc_tile_pool(name="recp", bufs=4)
            rec_dram = dramp.tile([H, NT], F32, name="rec_dram")
            for hp in range(H // 2):
                attn_head_pair(hp)
                if hp == 3:
                    normalize_batch(0)
            normalize_batch(1)

            def dbg_dump(nc, tc, dma):
                dbgp = tc.alloc_tile_pool(name="dbgp", bufs=2)
                for name, t_sb, t_d in ((
                    "qT", qT, dqT), ("kT", kT, dkT), ("attnT", attnT, dattnT)):
                    for ko in range(KI):
                        f = dbgp.tile([P, NT], F32, tag="dbgf", name=f"dbg_{name}{ko}")
                        nc.vector.tensor_copy(f[:], t_sb[:, ko])
                        dma(t_d.ap().rearrange("p k n -> p k n")[:, ko], f[:])
                for kb in range(KB):
                    f = dbgp.tile([P, H * (DH + 1)], F32, tag="dbgf", name=f"dbg_v{kb}")
                    nc.vector.tensor_copy(f[:].rearrange("p (h d) -> p h d", d=DH+1), vA[:, kb])
                    dma(dvA.ap()[:, kb], f[:].rearrange("p (h d) -> p h d", d=DH+1))
                for hp2 in range(H // 2):
                    f = dbgp.tile([P, NT], F32, tag="dbgf", name=f"dbg_av{hp2}")
                    nc.vector.tensor_copy(f[:], av_sb2[:, hp2])
                    dma(dav.ap()[:, hp2], f[:])
                dma(dsums.ap(), sums16[:])
                dma(drec.ap(), rec16[:])
                dbgp.release()


            av_sb2 = av_sb
            if dbg:
                dbg_dump(nc, tc, dma)
            recp.release()
            avp.release()
            expp.release()
            dramp.release()

            # ---------------- out projection + bias ----------------
            outp = tc.alloc_tile_pool(name="outp", bufs=2)
            out3 = out_d.ap().rearrange("(t p) d -> p t d", p=P)
            for mt in range(TB):
                ps = psA.tile([P, 1024], F32, tag="big")
                for n0 in (0, 512):
                    for kc in range(KI):
                        nc.tensor.matmul(
                            ps[:, n0:n0 + 512],
                            attnT[:, kc, mt * P:(mt + 1) * P],
                            wo_b[:, kc, n0:n0 + 512],
                            start=(kc == 0), stop=False)
                    nc.tensor.matmul(
                        ps[:, n0:n0 + 512],
                        ones_b[0:1, :],
                        bo_b[0:1, n0:n0 + 512],
                        start=False, stop=True)
                ot = outp.tile([P, DQ], F32, tag="out")
                nc.vector.tensor_copy(ot[:], ps[:])
                dma(out3[:, mt], ot[:])
            outp.release()

    nc.compile()
    return nc


_NC_CACHE = None


def _make_in_maps(inputs):
    x = np.ascontiguousarray(np.asarray(inputs["x"], dtype=np.float32))
    context = np.ascontiguousarray(np.asarray(inputs["context"], dtype=np.float32))
    shared = {
        "wq": np.ascontiguousarray(np.asarray(inputs["Wq"], dtype=np.float32)),
        "wk": np.ascontiguousarray(np.asarray(inputs["Wk"], dtype=np.float32)),
        "wv": np.ascontiguousarray(np.asarray(inputs["Wv"], dtype=np.float32)),
        "wo": np.ascontiguousarray(np.asarray(inputs["Wo"], dtype=np.float32)),
        "bo": np.ascontiguousarray(np.asarray(inputs["bo"], dtype=np.float32)),
    }
    in_maps = []
    for c in range(N_CORES):
        b, s = divmod(c, 2)
        in_maps.append({
            "x": np.ascontiguousarray(x[b, s * NT:(s + 1) * NT, :]),
            "ctx": np.ascontiguousarray(context[b]),
            **shared,
        })
    return in_maps


def kernel(x, context, Wq, Wk, Wv, Wo, bo):
    global _NC_CACHE
    if _NC_CACHE is None:
        _NC_CACHE = build()
    nc = _NC_CACHE

    in_maps = _make_in_maps(dict(x=x, context=context, Wq=Wq, Wk=Wk, Wv=Wv,
                                 Wo=Wo, bo=bo))
    res = run_bass_kernel_spmd(nc, in_maps, core_ids=list(range(N_CORES)))
    out = np.empty((B, NQ_FULL, DQ), dtype=np.float32)
    for c in range(N_CORES):
        b, s = divmod(c, 2)
        out[b, s * NT:(s + 1) * NT, :] = res.results[c]["out"]
    return out


# revision 28
# speedup vs baseline: 1.5732x; 1.0132x over previous
"""CrossAttention kernel for 8 TRN2 NeuronCores.

Sharding: 8 cores = 4 batches x 2 query-halves (zero communication).
Each core computes all 16 heads for its 1024 queries:
  q^T = Wq^T x^T, k^T = Wk^T ctx^T, v = ctx Wv          (bf16 matmuls)
  scores^T[kpos, q] = k^T.T q^T / 8                      (K=64 per head,
      even/odd head pairs issued adjacently -> PE row-group concurrency)
  exp on ScalarE straight from PSUM (no max subtraction; scores ~ N(0,1))
  attn_out^T[d, q] + denominators via ones-augmented V (M=65 matmuls)
  batched reciprocal of all 16 head denominators, per-head GPSIMD
  partition-broadcast, out-proj with bias as a K=1 accumulating matmul.
"""

import sys

for _p in ("/opt/trn_rl_repo", "/root/.axon_site/_ro/trn_rl_repo"):
    if _p not in sys.path:
        sys.path.append(_p)

import numpy as np

import concourse.bass as bass
import concourse.tile as tile
from concourse import bacc, mybir
from concourse.bass_utils import run_bass_kernel_spmd

F32 = mybir.dt.float32
BF16 = mybir.dt.bfloat16
EXP = mybir.ActivationFunctionType.Exp
MULT = mybir.AluOpType.mult

P = 128
B, NQ_FULL, DQ = 4, 2048, 1024
NK, DC = 1024, 768
H, DH = 16, 64
INNER = H * DH  # 1024
NT = 1024  # local queries per core
N_CORES = 8

KQ = DQ // P      # 8
KC = DC // P      # 6
KI = INNER // P   # 8
TB = NT // P      # 8
KB = NK // P      # 8
SCALE = 1.0 / np.sqrt(DH)


def build(dbg=False):
    nc = bacc.Bacc("TRN2", target_bir_lowering=False, debug=False,
                   enable_asserts=False, num_devices=N_CORES)

    x_d = nc.dram_tensor("x", [NT, DQ], F32, kind="ExternalInput")
    ctx_d = nc.dram_tensor("ctx", [NK, DC], F32, kind="ExternalInput")
    wq_d = nc.dram_tensor("wq", [DQ, INNER], F32, kind="ExternalInput")
    wk_d = nc.dram_tensor("wk", [DC, INNER], F32, kind="ExternalInput")
    wv_d = nc.dram_tensor("wv", [DC, INNER], F32, kind="ExternalInput")
    wo_d = nc.dram_tensor("wo", [INNER, DQ], F32, kind="ExternalInput")
    bo_d = nc.dram_tensor("bo", [DQ], F32, kind="ExternalInput")
    out_d = nc.dram_tensor("out", [NT, DQ], F32, kind="ExternalOutput")
    if dbg:
        dqT = nc.dram_tensor("dqT", [P, KI, NT], F32, kind="ExternalOutput")
        dkT = nc.dram_tensor("dkT", [P, KI, NK], F32, kind="ExternalOutput")
        dvA = nc.dram_tensor("dvA", [P, KB, H, DH + 1], F32, kind="ExternalOutput")
        dav = nc.dram_tensor("dav", [P, H // 2, NT], F32, kind="ExternalOutput")
        dsums = nc.dram_tensor("dsums", [H, NT], F32, kind="ExternalOutput")
        drec = nc.dram_tensor("drec", [H, NT], F32, kind="ExternalOutput")
        dattnT = nc.dram_tensor("dattnT", [P, KI, NT], F32, kind="ExternalOutput")

    dmae = [nc.sync, nc.scalar]  # HWDGE dispatchers, round-robined

    with tile.TileContext(nc) as tc:
        with (
            tc.tile_pool(name="persist", bufs=1) as persist,
            tc.tile_pool(name="psA", bufs=3, space="PSUM") as psA,
            tc.tile_pool(name="psV", bufs=2, space="PSUM") as psV,
        ):
            qT = persist.tile([P, KI, NT], BF16)     # [inner, q]
            kT = persist.tile([P, KI, NK], BF16)     # [inner, kpos]
            vA = persist.tile([P, KB, H, DH + 1], BF16)  # [kpos, (head, d|1)]
            attnT = persist.tile([P, KI, NT], BF16)  # [inner, q]
            wo_b = persist.tile([P, KI, DQ], BF16)
            bo_b = persist.tile([1, DQ], BF16)
            ones_b = persist.tile([1, P], BF16)

            dmai = 0

            def dma(out, in_):
                nonlocal dmai
                dmae[dmai % 2].dma_start(out, in_)
                dmai += 1

            def dmaT(out, in_):
                nc.sync.dma_start_transpose(out, in_)

            with tc.tile_pool(name="stage", bufs=1) as stage, \
                 tc.tile_pool(name="stage2", bufs=3) as stage2:
                xT = stage.tile([P, KQ, NT], BF16, tag="xT")
                cT = stage.tile([P, KC, NK], BF16, tag="cT")
                wq_b = stage.tile([P, KQ, INNER], BF16, tag="wq")
                wk_b = stage.tile([P, KC, INNER], BF16, tag="wk")
                wv_b = stage.tile([P, KC, INNER], BF16, tag="wv")

                nc.gpsimd.memset(ones_b[:], 1.0)
                nc.gpsimd.memset(vA[:, :, :, DH:DH + 1], 1.0)

                # x: load natural (2 token-blocks at a time), cast, big transpose
                x4 = x_d.ap().rearrange("(t p) d -> p t d", p=P)
                for t2 in range(0, TB, 2):
                    xf = stage2.tile([P, 2, DQ], F32, tag="ldf")
                    dma(xf[:], x4[:, t2:t2 + 2])
                    xb = stage2.tile([P, 2, DQ], BF16, tag="castb")
                    nc.vector.tensor_copy(xb[:], xf[:])
                    for t in (t2, t2 + 1):
                        dmaT(xT[:, :, t * P:(t + 1) * P], xb[:, t - t2])

                # wq
                wq4 = wq_d.ap().rearrange("(o p) m -> p o m", p=P)
                for ko in range(0, KQ, 2):
                    wf = stage2.tile([P, 2, INNER], F32, tag="ldf")
                    dma(wf[:], wq4[:, ko:ko + 2])
                    nc.scalar.copy(wq_b[:, ko:ko + 2], wf[:])

                # ---- q projection ----
                for ko in range(KI):
                    for n0 in range(0, NT, 512):
                        ps = psA.tile([P, 1024], F32, tag="big")
                        for kc in range(KQ):
                            nc.tensor.matmul(
                                ps[:, 0:512],
                                wq_b[:, kc, ko * P:(ko + 1) * P],
                                xT[:, kc, n0:n0 + 512],
                                start=(kc == 0), stop=(kc == KQ - 1))
                        nc.vector.tensor_copy(qT[:, ko, n0:n0 + 512],
                                              ps[:, 0:512])

                # ctx / wk / wv / wo / bo loads
                c4 = ctx_d.ap().rearrange("(t p) d -> p t d", p=P)
                for t2 in range(0, KB, 2):
                    cf = stage2.tile([P, 2, DQ], F32, tag="ldf")
                    dma(cf[:, :, :DC], c4[:, t2:t2 + 2])
                    cb = stage2.tile([P, 2, DQ], BF16, tag="castb")
                    nc.vector.tensor_copy(cb[:, :, :DC], cf[:, :, :DC])
                    for t in (t2, t2 + 1):
                        dmaT(cT[:, :, t * P:(t + 1) * P], cb[:, t - t2, :DC])
                wk4 = wk_d.ap().rearrange("(o p) m -> p o m", p=P)
                wv4 = wv_d.ap().rearrange("(o p) m -> p o m", p=P)
                for ko in range(0, KC, 2):
                    wf = stage2.tile([P, 2, INNER], F32, tag="ldf")
                    dma(wf[:], wk4[:, ko:ko + 2])
                    nc.scalar.copy(wk_b[:, ko:ko + 2], wf[:])
                    wf2 = stage2.tile([P, 2, INNER], F32, tag="ldf")
                    dma(wf2[:], wv4[:, ko:ko + 2])
                    nc.scalar.copy(wv_b[:, ko:ko + 2], wf2[:])
                wo4 = wo_d.ap().rearrange("(o p) m -> p o m", p=P)
                for ko in range(0, KI, 2):
                    wf = stage2.tile([P, 2, INNER], F32, tag="ldf")
                    dma(wf[:], wo4[:, ko:ko + 2])
                    nc.scalar.copy(wo_b[:, ko:ko + 2], wf[:])
                bo_f = stage.tile([1, DQ], F32, tag="bo")
                dma(bo_f[:], bo_d.ap()[None, :])
                nc.scalar.copy(bo_b[:], bo_f[:])

                # ---- k / v projections ----
                for ko in range(KI):
                    for n0 in range(0, NK, 512):
                        ps = psA.tile([P, 1024], F32, tag="big")
                        for kc in range(KC):
                            nc.tensor.matmul(
                                ps[:, 0:512],
                                wk_b[:, kc, ko * P:(ko + 1) * P],
                                cT[:, kc, n0:n0 + 512],
                                start=(kc == 0), stop=(kc == KC - 1))
                        nc.vector.tensor_copy(kT[:, ko, n0:n0 + 512],
                                              ps[:, 0:512])
                for mt in range(KB):
                    for n0 in range(0, INNER, 512):
                        ps = psA.tile([P, 1024], F32, tag="big")
                        for kc in range(KC):
                            nc.tensor.matmul(
                                ps[:, 0:512],
                                cT[:, kc, mt * P:(mt + 1) * P],
                                wv_b[:, kc, n0:n0 + 512],
                                start=(kc == 0), stop=(kc == KC - 1))
                        h0 = n0 // DH
                        nc.vector.tensor_copy(
                            vA[:, mt, h0:h0 + 8, 0:DH],
                            ps[:, 0:512].rearrange("p (h d) -> p h d", d=DH))

            # ---------------- attention, head pairs ----------------
            expp = tc.alloc_tile_pool(name="expp", bufs=16)
            avp = tc.alloc_tile_pool(name="avp", bufs=1)
            dramp = tc.alloc_tile_pool(name="dramp", bufs=1, space="DRAM")
            sums_dram = dramp.tile([H, NT], F32, name="sums_dram")
            av_sb = avp.tile([P, H // 2, NT], BF16, tag="avsb")  # [2*64d, hpair, q]
            def attn_head_pair(hp):
                h0, h1 = 2 * hp, 2 * hp + 1
                h2 = hp
                psvs = {h: [psV.tile([DH + 1, 512], F32, tag="av",
                                     name=f"psv{h}_{n}") for n in (0, 1)]
                        for h in (h0, h1)}
                ets_all = {h0: [], h1: []}
                for kb in range(KB):
                    pss = {}
                    for h in (h0, h1):
                        base = (h % 2) * DH
                        ps = psA.tile([P, 1024], F32, tag="big")
                        pss[h] = ps
                        for n0 in (0, 512):
                            nc.tensor.matmul(
                                ps[:, n0:n0 + 512],
                                kT[base:base + DH, h2, kb * P:(kb + 1) * P],
                                qT[base:base + DH, h2, n0:n0 + 512],
                                start=True, stop=True)
                    for h in (h0, h1):
                        et = expp.tile([P, NT], BF16, tag="exp")
                        nc.scalar.activation(et[:], pss[h][:], EXP,
                                             scale=float(SCALE))
                        ets_all[h].append(et)
                    for h in (h0, h1):
                        for ni, n0 in enumerate((0, 512)):
                            nc.tensor.matmul(
                                psvs[h][ni][:],
                                vA[:, kb, h, :],
                                ets_all[h][kb][:, n0:n0 + 512],
                                start=(kb == 0), stop=(kb == KB - 1))
                for i, h in enumerate((h0, h1)):
                    for ni, n0 in enumerate((0, 512)):
                        srow = expp.tile([1, 512], F32, tag="srow",
                                         name=f"srow{h}_{ni}")
                        nc.vector.tensor_copy(srow[:], psvs[h][ni][DH:DH + 1, :])
                        dma(sums_dram[h:h + 1, n0:n0 + 512], srow[:])
                        nc.vector.tensor_copy(
                            av_sb[i * DH:(i + 1) * DH, hp, n0:n0 + 512],
                            psvs[h][ni][0:DH, :])

            def normalize_batch(bi):
                sums8 = avp.tile([DH, P], F32, tag=f"sums{bi}",
                                 name=f"sums_b{bi}")
                dma(sums8[:], sums_dram[:]
                    .rearrange("h (a b) -> (h a) b", b=P)[bi * DH:(bi + 1) * DH])
                rec8 = avp.tile([DH, P], F32, tag=f"rec{bi}",
                                name=f"rec_b{bi}")
                nc.vector.reciprocal(rec8[:], sums8[:])
                dma(rec_dram[:]
                    .rearrange("h (a b) -> (h a) b", b=P)[bi * DH:(bi + 1) * DH],
                    rec8[:])
                for hp in range(bi * 4, (bi + 1) * 4):
                    rb = recp.tile([P, NT], F32, tag="rb")
                    for i in (0, 1):
                        src = rec_dram[2 * hp + i:2 * hp + i + 1, :]
                        bsrc = bass.AP(tensor=src.tensor, offset=src.offset,
                                       ap=[[0, DH]] + list(src.ap[1:]))
                        dma(rb[i * DH:(i + 1) * DH, :], bsrc)
                    nc.vector.tensor_tensor(attnT[:, hp, :],
                                            av_sb[:, hp, :],
                                            rb[:], MULT)

            recp = tc.alloc_tile_pool(name="recp", bufs=4)
            rec_dram = dramp.tile([H, NT], F32, name="rec_dram")
            for hp in range(H // 2):
                attn_head_pair(hp)
                if hp == 3:
                    normalize_batch(0)
            normalize_batch(1)

            def dbg_dump(nc, tc, dma):
                dbgp = tc.alloc_tile_pool(name="dbgp", bufs=2)
                for name, t_sb, t_d in ((
                    "qT", qT, dqT), ("kT", kT, dkT), ("attnT", attnT, dattnT)):
                    for ko in range(KI):
                        f = dbgp.tile([P, NT], F32, tag="dbgf", name=f"dbg_{name}{ko}")
                        nc.vector.tensor_copy(f[:], t_sb[:, ko])
                        dma(t_d.ap().rearrange("p k n -> p k n")[:, ko], f[:])
                for kb in range(KB):
                    f = dbgp.tile([P, H * (DH + 1)], F32, tag="dbgf", name=f"dbg_v{kb}")
                    nc.vector.tensor_copy(f[:].rearrange("p (h d) -> p h d", d=DH+1), vA[:, kb])
                    dma(dvA.ap()[:, kb], f[:].rearrange("p (h d) -> p h d", d=DH+1))
                for hp2 in range(H // 2):
                    f = dbgp.tile([P, NT], F32, tag="dbgf", name=f"dbg_av{hp2}")
                    nc.vector.tensor_copy(f[:], av_sb2[:, hp2])
                    dma(dav.ap()[:, hp2], f[:])
                dma(dsums.ap(), sums16[:])
                dma(drec.ap(), rec16[:])
                dbgp.release()


            av_sb2 = av_sb
            if dbg:
                dbg_dump(nc, tc, dma)
            recp.release()
            avp.release()
            expp.release()
            dramp.release()

            # ---------------- out projection + bias ----------------
            outp = tc.alloc_tile_pool(name="outp", bufs=2)
            out3 = out_d.ap().rearrange("(t p) d -> p t d", p=P)
            for mt in range(TB):
                ps = psA.tile([P, 1024], F32, tag="big")
                for n0 in (0, 512):
                    for kc in range(KI):
                        nc.tensor.matmul(
                            ps[:, n0:n0 + 512],
                            attnT[:, kc, mt * P:(mt + 1) * P],
                            wo_b[:, kc, n0:n0 + 512],
                            start=(kc == 0), stop=False)
                    nc.tensor.matmul(
                        ps[:, n0:n0 + 512],
                        ones_b[0:1, :],
                        bo_b[0:1, n0:n0 + 512],
                        start=False, stop=True)
                ot = outp.tile([P, DQ], F32, tag="out")
                nc.vector.tensor_copy(ot[:], ps[:])
                dma(out3[:, mt], ot[:])
            outp.release()

    nc.compile()
    return nc


_NC_CACHE = None


def _make_in_maps(inputs):
    x = np.ascontiguousarray(np.asarray(inputs["x"], dtype=np.float32))
    context = np.ascontiguousarray(np.asarray(inputs["context"], dtype=np.float32))
    shared = {
        "wq": np.ascontiguousarray(np.asarray(inputs["Wq"], dtype=np.float32)),
        "wk": np.ascontiguousarray(np.asarray(inputs["Wk"], dtype=np.float32)),
        "wv": np.ascontiguousarray(np.asarray(inputs["Wv"], dtype=np.float32)),
        "wo": np.ascontiguousarray(np.asarray(inputs["Wo"], dtype=np.float32)),
        "bo": np.ascontiguousarray(np.asarray(inputs["bo"], dtype=np.float32)),
    }
    in_maps = []
    for c in range(N_CORES):
        b, s = divmod(c, 2)
        in_maps.append({
            "x": np.ascontiguousarray(x[b, s * NT:(s + 1) * NT, :]),
            "ctx": np.ascontiguousarray(context[b]),
            **shared,
        })
    return in_maps


def kernel(x, context, Wq, Wk, Wv, Wo, bo):
    global _NC_CACHE
    if _NC_CACHE is None:
        _NC_CACHE = build()
    nc = _NC_CACHE

    in_maps = _make_in_maps(dict(x=x, context=context, Wq=Wq, Wk=Wk, Wv=Wv,
                                 Wo=Wo, bo=bo))
    res = run_bass_kernel_spmd(nc, in_maps, core_ids=list(range(N_CORES)))
    out = np.empty((B, NQ_FULL, DQ), dtype=np.float32)
    for c in range(N_CORES):
        b, s = divmod(c, 2)
        out[b, s * NT:(s + 1) * NT, :] = res.results[c]["out"]
    return out
